# revision 7
# baseline (speedup 1.0000x reference)
"""FFT depthwise conv == direct 7x7 circular depthwise conv, on 8 TRN2 cores.

out[b,i,j,c] = sum_{u,v} wf[c,u,v] * x[b,(i+u-3)%H,(j+v-3)%W,c],  wf = kernel[:, ::-1, ::-1]

v2: all 49 taps on TensorE via banded-Toeplitz matmuls.

Sharding: data-parallel over batch (1 image per core). Per channel-pair
(2 channels x 64-row W-windows on the 128 partitions), the v-convolution
is one matmul with a block-diagonal banded-Toeplitz stationary matrix
T[p, m] = wf[c, u, p-m]; the 7 u-taps accumulate in PSUM fp32. Input is
host-side transposed per channel (partitions = padded W, free = padded H),
so tap u is just a free-dim offset into the same SBUF tile.

Per (pair g, window t): 7 matmuls N=224 -> psum[128, 224]; valid output
rows m in [m0_t, m0_t+nj_t) map to out columns j = W0S[t] + m for ch 2g
(partitions m) and 2g+1 (partitions 64+m). DVE/ScalarE alternate the
psum -> bf16 staging copies; host undoes all layout shuffling.

DMA: host pre-tiles x/w/out into [96, 128, free] tensors so each 8-pair
block moves as ONE dma_start of 1024 contiguous ~1.8KB descriptors --
that granularity fans out across all 16 DMA engines (~370 GB/s measured
vs ~50 GB/s for 128-descriptor batches). x+w blocks on the sync queue,
out blocks on gpsimd, leaving ScalarE/DVE free for the psum copies.
"""

import os
import sys

for _p in ("/opt/trn_rl_repo", "/root/.axon_site/_ro/trn_rl_repo"):
    if os.path.isdir(_p) and _p not in sys.path:
        sys.path.insert(0, _p)

import numpy as np

import concourse.bacc as bacc
import concourse.bass as bass
import concourse.mybir as mybir
from concourse.bass_utils import run_bass_kernel_spmd
from concourse.tile import TileContext

F32 = mybir.dt.float32
BF16 = mybir.dt.bfloat16

B, H, W, C, K = 8, 224, 224, 192, 7
NCORES = 8
PAD = K // 2                  # 3
HP = H + 2 * PAD              # 230 padded rows (free dim)
WP = W + 2 * PAD              # 230 padded cols (partition windows)
NPAIR = C // 2                # 96 channel pairs
W0S = [0, 58, 116, 166]       # window starts (input w-padded coords)
M0S = [0, 0, 0, 8]            # first valid m per window
NJS = [58, 58, 58, 50]        # valid rows per window
NT = len(W0S)                 # 4 windows
XF = NT * HP                  # 920 x free elems per (pair, partition)
WF = K * 128                  # 896 w free elems
OF = NT * H                   # 896 out free elems
BLK = 8                       # max pairs per DMA block
# small edge blocks (with fine-grained DMA descriptors) shrink the startup
# wait before the first matmul and the drain after the last one
PB = [2, 4] + [8] * 11 + [2]
assert sum(PB) == NPAIR


def build_nc():
    nc = bacc.Bacc()
    x_d = nc.declare_dram_parameter("x", [NPAIR, 128, XF], BF16, isOutput=False)
    w_d = nc.declare_dram_parameter("w", [NPAIR, 128, WF], BF16, isOutput=False)
    o_d = nc.declare_dram_parameter("out", [NPAIR, 128, OF], BF16, isOutput=True)
    xh = x_d.tensor if hasattr(x_d, "tensor") else x_d
    wh = w_d.tensor if hasattr(w_d, "tensor") else w_d
    oh = o_d.tensor if hasattr(o_d, "tensor") else o_d

    def xap(g0, n, fine):
        dims = (
            [[XF, 128], [128 * XF, n], [HP, NT], [1, HP]]
            if fine
            else [[XF, 128], [128 * XF, n], [1, XF]]
        )
        return bass.AP(xh, g0 * 128 * XF, dims)

    def wap(g0, n, fine):
        dims = (
            [[WF, 128], [128 * WF, n], [128, K], [1, 128]]
            if fine
            else [[WF, 128], [128 * WF, n], [1, WF]]
        )
        return bass.AP(wh, g0 * 128 * WF, dims)

    def oap(g0, n, fine):
        dims = (
            [[OF, 128], [128 * OF, n], [H, NT], [1, H]]
            if fine
            else [[OF, 128], [128 * OF, n], [1, OF]]
        )
        return bass.AP(oh, g0 * 128 * OF, dims)

    with TileContext(nc) as tc:
        with (
            tc.tile_pool(name="xin", bufs=3) as xpool,
            tc.tile_pool(name="wts", bufs=3) as wpool,
            tc.tile_pool(name="outp", bufs=3) as opool,
            tc.tile_pool(name="psum", bufs=8, space="PSUM") as ppool,
        ):
            # pre-warm the PE clock gate (HAM) with throwaway matmuls so
            # the real stream starts at 2.4 GHz instead of 1.2 GHz
            warm = xpool.tile([128, 64], BF16, name="warm", tag="warm")
            nc.vector.memset(warm[:], 1.0)
            wps = ppool.tile([128, 512], F32, name="warmps", tag="ps")
            NWARM = 30
            for i in range(NWARM):
                nc.tensor.matmul(
                    wps[0:64, 0:64],
                    warm[:, 0:64],
                    warm[:, 0:64],
                    start=(i == 0),
                    stop=(i == NWARM - 1),
                )

            g0 = 0
            for blk, n in enumerate(PB):
                fine = n <= 2
                xt = xpool.tile([128, BLK, XF], BF16, name=f"x{blk}", tag="x")
                wt = wpool.tile([128, BLK, WF], BF16, name=f"w{blk}", tag="w")
                if blk <= 1:
                    # startup ramp: land pairs in 2-pair fine-grained chunks
                    # so the first matmuls gate on ~0.5MB, not the full block
                    step = 1 if blk == 0 else 2
                    for p0 in range(0, n, step):
                        nc.sync.dma_start(
                            out=xt[:, p0:p0 + step, :].rearrange(
                                "p b (t h) -> p b t h", t=NT
                            ),
                            in_=xap(g0 + p0, step, True),
                        )
                        nc.gpsimd.dma_start(
                            out=wt[:, p0:p0 + step, :].rearrange(
                                "p b (u m) -> p b u m", u=K
                            ),
                            in_=wap(g0 + p0, step, True),
                        )
                else:
                    nc.sync.dma_start(
                        out=(
                            xt[:, 0:n, :].rearrange("p b (t h) -> p b t h", t=NT)
                            if fine
                            else xt[:, 0:n, :]
                        ),
                        in_=xap(g0, n, fine),
                    )
                    nc.gpsimd.dma_start(
                        out=(
                            wt[:, 0:n, :].rearrange("p b (u m) -> p b u m", u=K)
                            if fine
                            else wt[:, 0:n, :]
                        ),
                        in_=wap(g0, n, fine),
                    )
                ot = opool.tile([128, BLK, OF], BF16, name=f"o{blk}", tag="o")
                for pl in range(n):
                    for t in range(NT):
                        ps = ppool.tile(
                            [128, 512], F32, name=f"ps{blk}_{pl}_{t}", tag="ps"
                        )
                        for u in range(K):
                            nc.tensor.matmul(
                                ps[:, 0:H],
                                wt[:, pl, u * 128:(u + 1) * 128],
                                xt[:, pl, t * HP + u:t * HP + u + H],
                                start=(u == 0),
                                stop=(u == K - 1),
                            )
                        nc.vector.tensor_scalar_mul(
                            ot[:, pl, t * H:(t + 1) * H], ps[:, 0:H], 1.0
                        )
                nc.scalar.dma_start(
                    out=oap(g0, n, fine),
                    in_=(
                        ot[:, 0:n, :].rearrange("p b (t h) -> p b t h", t=NT)
                        if fine
                        else ot[:, 0:n, :]
                    ),
                )
                g0 += n
    return nc


def _host_x(x):
    """x: (B, H, W, C) f32 -> (B, NPAIR, 128, XF) bf16 pre-tiled."""
    import ml_dtypes

    # xT[b, c, wp, hp] = x[b, (hp-3)%H, (wp-3)%W, c]
    xT = np.ascontiguousarray(x.transpose(0, 3, 2, 1))          # (B, C, W, H)
    xT = np.pad(xT, ((0, 0), (0, 0), (PAD, PAD), (PAD, PAD)), mode="wrap")
    xT = xT.astype(ml_dtypes.bfloat16)                          # (B, C, WP, HP)
    xdev = np.empty((B, NPAIR, 128, NT, HP), dtype=ml_dtypes.bfloat16)
    for t, w0 in enumerate(W0S):
        win = xT[:, :, w0:w0 + 64, :]                           # (B, C, 64, HP)
        xdev[:, :, 0:64, t, :] = win[:, 0::2]
        xdev[:, :, 64:128, t, :] = win[:, 1::2]
    return xdev.reshape(B, NPAIR, 128, XF)


def _host_w(kernel):
    """kernel: (C, K, K) -> (NPAIR, 128, WF) bf16 block-diag Toeplitz."""
    import ml_dtypes

    wf = kernel[:, ::-1, ::-1].astype(np.float32)               # flipped taps
    wdev = np.zeros((NPAIR, 128, K, 128), dtype=np.float32)
    m = np.arange(64)
    for v in range(K):
        pm = m + v                                              # p = m + v
        ok = pm < 64
        # block 0: channel 2g, block 1: channel 2g+1
        # advanced indices (pm, m) land in front: result (ndiag, NPAIR, K)
        wdev[:, pm[ok], :, m[ok]] = wf[0::2, :, v][None, :, :]
        wdev[:, 64 + pm[ok], :, 64 + m[ok]] = wf[1::2, :, v][None, :, :]
    return np.ascontiguousarray(wdev.reshape(NPAIR, 128, WF)).astype(
        ml_dtypes.bfloat16
    )


def _host_unshuffle(odev):
    """odev: (B, NPAIR, 128, OF) -> (B, H, W, C) f32."""
    o = np.asarray(odev, dtype=np.float32).reshape(B, NPAIR, 128, NT, H)
    out = np.empty((B, H, W, C), dtype=np.float32)
    for t, w0 in enumerate(W0S):
        m0, nj = M0S[t], NJS[t]
        j0 = w0 + m0
        # o[b, g, m, t, i] -> out[b, i, j0+mm, 2g]; partitions 64+m -> 2g+1
        out[:, :, j0:j0 + nj, 0::2] = o[:, :, m0:m0 + nj, t, :].transpose(0, 3, 2, 1)
        out[:, :, j0:j0 + nj, 1::2] = o[:, :, 64 + m0:64 + m0 + nj, t, :].transpose(
            0, 3, 2, 1
        )
    return out


_NC_CACHE = {}


def _get_nc():
    if "nc" not in _NC_CACHE:
        nc = build_nc()
        nc.finalize()
        _NC_CACHE["nc"] = nc
    return _NC_CACHE["nc"]


def run(x, kernel, trace=False, **kw):
    assert x.shape == (B, H, W, C) and kernel.shape == (C, K, K)
    nc = _get_nc()
    xdev = _host_x(np.asarray(x, dtype=np.float32))
    wdev = _host_w(np.asarray(kernel))
    in_maps = [{"x": xdev[b], "w": wdev} for b in range(NCORES)]
    res = run_bass_kernel_spmd(nc, in_maps, list(range(NCORES)), trace=trace, **kw)
    odev = np.stack([np.asarray(res.results[b]["out"]) for b in range(NCORES)])
    return _host_unshuffle(odev), res


def kernel(x, kernel):
    out, _ = run(np.asarray(x), np.asarray(kernel))
    return out


# revision 8
# speedup vs baseline: 1.0429x; 1.0429x over previous
"""FFT depthwise conv == direct 7x7 circular depthwise conv, on 8 TRN2 cores.

out[b,i,j,c] = sum_{u,v} wf[c,u,v] * x[b,(i+u-3)%H,(j+v-3)%W,c],  wf = kernel[:, ::-1, ::-1]

v2: all 49 taps on TensorE via banded-Toeplitz matmuls.

Sharding: data-parallel over batch (1 image per core). Per channel-pair
(2 channels x 64-row W-windows on the 128 partitions), the v-convolution
is one matmul with a block-diagonal banded-Toeplitz stationary matrix
T[p, m] = wf[c, u, p-m]; the 7 u-taps accumulate in PSUM fp32. Input is
host-side transposed per channel (partitions = padded W, free = padded H),
so tap u is just a free-dim offset into the same SBUF tile.

Per (pair g, window t): 7 matmuls N=224 -> psum[128, 224]; valid output
rows m in [m0_t, m0_t+nj_t) map to out columns j = W0S[t] + m for ch 2g
(partitions m) and 2g+1 (partitions 64+m). DVE/ScalarE alternate the
psum -> bf16 staging copies; host undoes all layout shuffling.

DMA: host pre-tiles x/w/out into [96, 128, free] tensors so each 8-pair
block moves as ONE dma_start of 1024 contiguous ~1.8KB descriptors --
that granularity fans out across all 16 DMA engines (~370 GB/s measured
vs ~50 GB/s for 128-descriptor batches). x+w blocks on the sync queue,
out blocks on gpsimd, leaving ScalarE/DVE free for the psum copies.
"""

import os
import sys

for _p in ("/opt/trn_rl_repo", "/root/.axon_site/_ro/trn_rl_repo"):
    if os.path.isdir(_p) and _p not in sys.path:
        sys.path.insert(0, _p)

import numpy as np

import concourse.bacc as bacc
import concourse.bass as bass
import concourse.mybir as mybir
from concourse.bass_utils import run_bass_kernel_spmd
from concourse.tile import TileContext

F32 = mybir.dt.float32
BF16 = mybir.dt.bfloat16

B, H, W, C, K = 8, 224, 224, 192, 7
NCORES = 8
PAD = K // 2                  # 3
HP = H + 2 * PAD              # 230 padded rows (free dim)
WP = W + 2 * PAD              # 230 padded cols (partition windows)
NPAIR = C // 2                # 96 channel pairs
W0S = [0, 58, 116, 166]       # window starts (input w-padded coords)
M0S = [0, 0, 0, 8]            # first valid m per window
NJS = [58, 58, 58, 50]        # valid rows per window
NT = len(W0S)                 # 4 windows
XF = NT * HP                  # 920 x free elems per (pair, partition)
WF = K * 128                  # 896 w free elems
OF = NT * H                   # 896 out free elems
BLK = 8                       # max pairs per DMA block
# small edge blocks (with fine-grained DMA descriptors) shrink the startup
# wait before the first matmul and the drain after the last one
PB = [8] * 12
assert sum(PB) == NPAIR


def build_nc():
    nc = bacc.Bacc()
    x_d = nc.declare_dram_parameter("x", [NPAIR, 128, XF], BF16, isOutput=False)
    w_d = nc.declare_dram_parameter("w", [NPAIR, 128, WF], BF16, isOutput=False)
    o_d = nc.declare_dram_parameter("out", [NPAIR, 128, OF], BF16, isOutput=True)
    xh = x_d.tensor if hasattr(x_d, "tensor") else x_d
    wh = w_d.tensor if hasattr(w_d, "tensor") else w_d
    oh = o_d.tensor if hasattr(o_d, "tensor") else o_d

    def xap(g0, n, fine):
        dims = (
            [[XF, 128], [128 * XF, n], [HP, NT], [1, HP]]
            if fine
            else [[XF, 128], [128 * XF, n], [1, XF]]
        )
        return bass.AP(xh, g0 * 128 * XF, dims)

    def wap(g0, n, fine):
        dims = (
            [[WF, 128], [128 * WF, n], [128, K], [1, 128]]
            if fine
            else [[WF, 128], [128 * WF, n], [1, WF]]
        )
        return bass.AP(wh, g0 * 128 * WF, dims)

    def oap(g0, n, fine):
        dims = (
            [[OF, 128], [128 * OF, n], [H, NT], [1, H]]
            if fine
            else [[OF, 128], [128 * OF, n], [1, OF]]
        )
        return bass.AP(oh, g0 * 128 * OF, dims)

    with TileContext(nc) as tc:
        with (
            tc.tile_pool(name="xin", bufs=2) as xpool,
            tc.tile_pool(name="wts", bufs=2) as wpool,
            tc.tile_pool(name="outp", bufs=2) as opool,
            tc.tile_pool(name="psum", bufs=8, space="PSUM") as ppool,
        ):
            # pre-warm the PE clock gate (HAM) with throwaway matmuls so
            # the real stream starts at 2.4 GHz instead of 1.2 GHz
            warm = xpool.tile([128, 64], BF16, name="warm", tag="warm")
            nc.vector.memset(warm[:], 1.0)
            wps = ppool.tile([128, 512], F32, name="warmps", tag="ps")
            NWARM = 30
            for i in range(NWARM):
                nc.tensor.matmul(
                    wps[0:64, 0:64],
                    warm[:, 0:64],
                    warm[:, 0:64],
                    start=(i == 0),
                    stop=(i == NWARM - 1),
                )

            g0 = 0
            for blk, n in enumerate(PB):
                xt = xpool.tile([128, BLK, XF], BF16, name=f"x{blk}", tag="x")
                wt = wpool.tile([128, BLK, WF], BF16, name=f"w{blk}", tag="w")
                nc.sync.dma_start(out=xt[:, 0:n, :], in_=xap(g0, n, False))
                nc.gpsimd.dma_start(out=wt[:, 0:n, :], in_=wap(g0, n, False))
                ot = opool.tile([128, BLK, OF], BF16, name=f"o{blk}", tag="o")
                for pl in range(n):
                    for t in range(NT):
                        ps = ppool.tile(
                            [128, 512], F32, name=f"ps{blk}_{pl}_{t}", tag="ps"
                        )
                        for u in range(K):
                            nc.tensor.matmul(
                                ps[:, 0:H],
                                wt[:, pl, u * 128:(u + 1) * 128],
                                xt[:, pl, t * HP + u:t * HP + u + H],
                                start=(u == 0),
                                stop=(u == K - 1),
                            )
                        nc.vector.tensor_scalar_mul(
                            ot[:, pl, t * H:(t + 1) * H], ps[:, 0:H], 1.0
                        )
                if blk == len(PB) - 1:
                    # final block: drain in 2-pair fine chunks so the last
                    # DMA after the last copy is small
                    for p0 in range(0, n, 2):
                        nc.scalar.dma_start(
                            out=oap(g0 + p0, 2, True),
                            in_=ot[:, p0:p0 + 2, :].rearrange(
                                "p b (t h) -> p b t h", t=NT
                            ),
                        )
                else:
                    nc.scalar.dma_start(out=oap(g0, n, False), in_=ot[:, 0:n, :])
                g0 += n
    return nc


def _host_x(x):
    """x: (B, H, W, C) f32 -> (B, NPAIR, 128, XF) bf16 pre-tiled."""
    import ml_dtypes

    # xT[b, c, wp, hp] = x[b, (hp-3)%H, (wp-3)%W, c]
    xT = np.ascontiguousarray(x.transpose(0, 3, 2, 1))          # (B, C, W, H)
    xT = np.pad(xT, ((0, 0), (0, 0), (PAD, PAD), (PAD, PAD)), mode="wrap")
    xT = xT.astype(ml_dtypes.bfloat16)                          # (B, C, WP, HP)
    xdev = np.empty((B, NPAIR, 128, NT, HP), dtype=ml_dtypes.bfloat16)
    for t, w0 in enumerate(W0S):
        win = xT[:, :, w0:w0 + 64, :]                           # (B, C, 64, HP)
        xdev[:, :, 0:64, t, :] = win[:, 0::2]
        xdev[:, :, 64:128, t, :] = win[:, 1::2]
    return xdev.reshape(B, NPAIR, 128, XF)


def _host_w(kernel):
    """kernel: (C, K, K) -> (NPAIR, 128, WF) bf16 block-diag Toeplitz."""
    import ml_dtypes

    wf = kernel[:, ::-1, ::-1].astype(np.float32)               # flipped taps
    wdev = np.zeros((NPAIR, 128, K, 128), dtype=np.float32)
    m = np.arange(64)
    for v in range(K):
        pm = m + v                                              # p = m + v
        ok = pm < 64
        # block 0: channel 2g, block 1: channel 2g+1
        # advanced indices (pm, m) land in front: result (ndiag, NPAIR, K)
        wdev[:, pm[ok], :, m[ok]] = wf[0::2, :, v][None, :, :]
        wdev[:, 64 + pm[ok], :, 64 + m[ok]] = wf[1::2, :, v][None, :, :]
    return np.ascontiguousarray(wdev.reshape(NPAIR, 128, WF)).astype(
        ml_dtypes.bfloat16
    )


def _host_unshuffle(odev):
    """odev: (B, NPAIR, 128, OF) -> (B, H, W, C) f32."""
    o = np.asarray(odev, dtype=np.float32).reshape(B, NPAIR, 128, NT, H)
    out = np.empty((B, H, W, C), dtype=np.float32)
    for t, w0 in enumerate(W0S):
        m0, nj = M0S[t], NJS[t]
        j0 = w0 + m0
        # o[b, g, m, t, i] -> out[b, i, j0+mm, 2g]; partitions 64+m -> 2g+1
        out[:, :, j0:j0 + nj, 0::2] = o[:, :, m0:m0 + nj, t, :].transpose(0, 3, 2, 1)
        out[:, :, j0:j0 + nj, 1::2] = o[:, :, 64 + m0:64 + m0 + nj, t, :].transpose(
            0, 3, 2, 1
        )
    return out


_NC_CACHE = {}


def _get_nc():
    if "nc" not in _NC_CACHE:
        nc = build_nc()
        nc.finalize()
        _NC_CACHE["nc"] = nc
    return _NC_CACHE["nc"]


def run(x, kernel, trace=False, **kw):
    assert x.shape == (B, H, W, C) and kernel.shape == (C, K, K)
    nc = _get_nc()
    xdev = _host_x(np.asarray(x, dtype=np.float32))
    wdev = _host_w(np.asarray(kernel))
    in_maps = [{"x": xdev[b], "w": wdev} for b in range(NCORES)]
    res = run_bass_kernel_spmd(nc, in_maps, list(range(NCORES)), trace=trace, **kw)
    odev = np.stack([np.asarray(res.results[b]["out"]) for b in range(NCORES)])
    return _host_unshuffle(odev), res


def kernel(x, kernel):
    out, _ = run(np.asarray(x), np.asarray(kernel))
    return out


# revision 9
# speedup vs baseline: 1.0573x; 1.0137x over previous
"""FFT depthwise conv == direct 7x7 circular depthwise conv, on 8 TRN2 cores.

out[b,i,j,c] = sum_{u,v} wf[c,u,v] * x[b,(i+u-3)%H,(j+v-3)%W,c],  wf = kernel[:, ::-1, ::-1]

v2: all 49 taps on TensorE via banded-Toeplitz matmuls.

Sharding: data-parallel over batch (1 image per core). Per channel-pair
(2 channels x 64-row W-windows on the 128 partitions), the v-convolution
is one matmul with a block-diagonal banded-Toeplitz stationary matrix
T[p, m] = wf[c, u, p-m]; the 7 u-taps accumulate in PSUM fp32. Input is
host-side transposed per channel (partitions = padded W, free = padded H),
so tap u is just a free-dim offset into the same SBUF tile.

Per (pair g, window t): 7 matmuls N=224 -> psum[128, 224]; valid output
rows m in [m0_t, m0_t+nj_t) map to out columns j = W0S[t] + m for ch 2g
(partitions m) and 2g+1 (partitions 64+m). DVE/ScalarE alternate the
psum -> bf16 staging copies; host undoes all layout shuffling.

DMA: host pre-tiles x/w/out into [96, 128, free] tensors so each 8-pair
block moves as ONE dma_start of 1024 contiguous ~1.8KB descriptors --
that granularity fans out across all 16 DMA engines (~370 GB/s measured
vs ~50 GB/s for 128-descriptor batches). x+w blocks on the sync queue,
out blocks on gpsimd, leaving ScalarE/DVE free for the psum copies.
"""

import os
import sys

for _p in ("/opt/trn_rl_repo", "/root/.axon_site/_ro/trn_rl_repo"):
    if os.path.isdir(_p) and _p not in sys.path:
        sys.path.insert(0, _p)

import numpy as np

import concourse.bacc as bacc
import concourse.bass as bass
import concourse.mybir as mybir
from concourse.bass_utils import run_bass_kernel_spmd
from concourse.tile import TileContext

F32 = mybir.dt.float32
BF16 = mybir.dt.bfloat16

B, H, W, C, K = 8, 224, 224, 192, 7
NCORES = 8
PAD = K // 2                  # 3
HP = H + 2 * PAD              # 230 padded rows (free dim)
WP = W + 2 * PAD              # 230 padded cols (partition windows)
NPAIR = C // 2                # 96 channel pairs
W0S = [0, 58, 116, 166]       # window starts (input w-padded coords)
M0S = [0, 0, 0, 8]            # first valid m per window
NJS = [58, 58, 58, 50]        # valid rows per window
NT = len(W0S)                 # 4 windows
XF = NT * HP                  # 920 x free elems per (pair, partition)
WF = K * 128                  # 896 w free elems
OF = NT * H                   # 896 out free elems
BLK = 8                       # max pairs per DMA block
# small edge blocks (with fine-grained DMA descriptors) shrink the startup
# wait before the first matmul and the drain after the last one
# startup ramp: one-off tiles (no pool rotation, no sub-slice writers) so
# the first matmul gates on ~0.5MB; uniform 8-pair rotating blocks after
RAMP = [1, 1, 2, 2, 4]
PB = [8] * 10 + [6]
assert sum(RAMP) + sum(PB) == NPAIR


def build_nc():
    nc = bacc.Bacc()
    x_d = nc.declare_dram_parameter("x", [NPAIR, 128, XF], BF16, isOutput=False)
    w_d = nc.declare_dram_parameter("w", [NPAIR, 128, WF], BF16, isOutput=False)
    o_d = nc.declare_dram_parameter("out", [NPAIR, 128, OF], BF16, isOutput=True)
    xh = x_d.tensor if hasattr(x_d, "tensor") else x_d
    wh = w_d.tensor if hasattr(w_d, "tensor") else w_d
    oh = o_d.tensor if hasattr(o_d, "tensor") else o_d

    def xap(g0, n, fine):
        dims = (
            [[XF, 128], [128 * XF, n], [HP, NT], [1, HP]]
            if fine
            else [[XF, 128], [128 * XF, n], [1, XF]]
        )
        return bass.AP(xh, g0 * 128 * XF, dims)

    def wap(g0, n, fine):
        dims = (
            [[WF, 128], [128 * WF, n], [128, K], [1, 128]]
            if fine
            else [[WF, 128], [128 * WF, n], [1, WF]]
        )
        return bass.AP(wh, g0 * 128 * WF, dims)

    def oap(g0, n, fine):
        dims = (
            [[OF, 128], [128 * OF, n], [H, NT], [1, H]]
            if fine
            else [[OF, 128], [128 * OF, n], [1, OF]]
        )
        return bass.AP(oh, g0 * 128 * OF, dims)

    with TileContext(nc) as tc:
        with (
            tc.tile_pool(name="ramp", bufs=1) as rpool,
            tc.tile_pool(name="xin", bufs=2) as xpool,
            tc.tile_pool(name="wts", bufs=2) as wpool,
            tc.tile_pool(name="outp", bufs=2) as opool,
            tc.tile_pool(name="psum", bufs=8, space="PSUM") as ppool,
        ):
            # pre-warm the PE clock gate (HAM) with throwaway matmuls so
            # the real stream starts at 2.4 GHz instead of 1.2 GHz
            warm = rpool.tile([128, 128], BF16, name="warm", tag="warm")
            nc.vector.memset(warm[:], 1.0)
            wps = ppool.tile([128, 512], F32, name="warmps", tag="ps")
            NWARM = 18
            for i in range(NWARM):
                nc.tensor.matmul(
                    wps[0:128, 0:128],
                    warm[:, 0:128],
                    warm[:, 0:128],
                    start=(i == 0),
                    stop=(i == NWARM - 1),
                )

            def compute(xt, wt, ot, n, blki):
                for pl in range(n):
                    for t in range(NT):
                        ps = ppool.tile(
                            [128, 512], F32, name=f"ps{blki}_{pl}_{t}", tag="ps"
                        )
                        for u in range(K):
                            nc.tensor.matmul(
                                ps[:, 0:H],
                                wt[:, pl, u * 128:(u + 1) * 128],
                                xt[:, pl, t * HP + u:t * HP + u + H],
                                start=(u == 0),
                                stop=(u == K - 1),
                            )
                        nc.vector.tensor_scalar_mul(
                            ot[:, pl, t * H:(t + 1) * H], ps[:, 0:H], 1.0
                        )

            g0 = 0
            for ri, n in enumerate(RAMP):
                fine = n <= 2
                xt = rpool.tile([128, n, XF], BF16, name=f"rx{ri}", tag=f"rx{ri}")
                nc.sync.dma_start(
                    out=(
                        xt[:, :, :].rearrange("p b (t h) -> p b t h", t=NT)
                        if fine
                        else xt[:, :, :]
                    ),
                    in_=xap(g0, n, fine),
                )
                wt = rpool.tile([128, n, WF], BF16, name=f"rw{ri}", tag=f"rw{ri}")
                nc.gpsimd.dma_start(
                    out=(
                        wt[:, :, :].rearrange("p b (u m) -> p b u m", u=K)
                        if fine
                        else wt[:, :, :]
                    ),
                    in_=wap(g0, n, fine),
                )
                ot = rpool.tile([128, n, OF], BF16, name=f"ro{ri}", tag=f"ro{ri}")
                compute(xt, wt, ot, n, f"r{ri}")
                nc.scalar.dma_start(
                    out=oap(g0, n, fine),
                    in_=(
                        ot[:, :, :].rearrange("p b (t h) -> p b t h", t=NT)
                        if fine
                        else ot[:, :, :]
                    ),
                )
                g0 += n

            for blk, n in enumerate(PB):
                xt = xpool.tile([128, BLK, XF], BF16, name=f"x{blk}", tag="x")
                wt = wpool.tile([128, BLK, WF], BF16, name=f"w{blk}", tag="w")
                nc.sync.dma_start(out=xt[:, 0:n, :], in_=xap(g0, n, False))
                nc.gpsimd.dma_start(out=wt[:, 0:n, :], in_=wap(g0, n, False))
                ot = opool.tile([128, BLK, OF], BF16, name=f"o{blk}", tag="o")
                compute(xt, wt, ot, n, blk)
                if blk == len(PB) - 1:
                    # final block: drain in 2-pair fine chunks so the last
                    # DMA after the last copy is small
                    for p0 in range(0, n, 2):
                        nc.scalar.dma_start(
                            out=oap(g0 + p0, 2, True),
                            in_=ot[:, p0:p0 + 2, :].rearrange(
                                "p b (t h) -> p b t h", t=NT
                            ),
                        )
                else:
                    nc.scalar.dma_start(out=oap(g0, n, False), in_=ot[:, 0:n, :])
                g0 += n
    return nc


def _host_x(x):
    """x: (B, H, W, C) f32 -> (B, NPAIR, 128, XF) bf16 pre-tiled."""
    import ml_dtypes

    # xT[b, c, wp, hp] = x[b, (hp-3)%H, (wp-3)%W, c]
    xT = np.ascontiguousarray(x.transpose(0, 3, 2, 1))          # (B, C, W, H)
    xT = np.pad(xT, ((0, 0), (0, 0), (PAD, PAD), (PAD, PAD)), mode="wrap")
    xT = xT.astype(ml_dtypes.bfloat16)                          # (B, C, WP, HP)
    xdev = np.empty((B, NPAIR, 128, NT, HP), dtype=ml_dtypes.bfloat16)
    for t, w0 in enumerate(W0S):
        win = xT[:, :, w0:w0 + 64, :]                           # (B, C, 64, HP)
        xdev[:, :, 0:64, t, :] = win[:, 0::2]
        xdev[:, :, 64:128, t, :] = win[:, 1::2]
    return xdev.reshape(B, NPAIR, 128, XF)


def _host_w(kernel):
    """kernel: (C, K, K) -> (NPAIR, 128, WF) bf16 block-diag Toeplitz."""
    import ml_dtypes

    wf = kernel[:, ::-1, ::-1].astype(np.float32)               # flipped taps
    wdev = np.zeros((NPAIR, 128, K, 128), dtype=np.float32)
    m = np.arange(64)
    for v in range(K):
        pm = m + v                                              # p = m + v
        ok = pm < 64
        # block 0: channel 2g, block 1: channel 2g+1
        # advanced indices (pm, m) land in front: result (ndiag, NPAIR, K)
        wdev[:, pm[ok], :, m[ok]] = wf[0::2, :, v][None, :, :]
        wdev[:, 64 + pm[ok], :, 64 + m[ok]] = wf[1::2, :, v][None, :, :]
    return np.ascontiguousarray(wdev.reshape(NPAIR, 128, WF)).astype(
        ml_dtypes.bfloat16
    )


def _host_unshuffle(odev):
    """odev: (B, NPAIR, 128, OF) -> (B, H, W, C) f32."""
    o = np.asarray(odev, dtype=np.float32).reshape(B, NPAIR, 128, NT, H)
    out = np.empty((B, H, W, C), dtype=np.float32)
    for t, w0 in enumerate(W0S):
        m0, nj = M0S[t], NJS[t]
        j0 = w0 + m0
        # o[b, g, m, t, i] -> out[b, i, j0+mm, 2g]; partitions 64+m -> 2g+1
        out[:, :, j0:j0 + nj, 0::2] = o[:, :, m0:m0 + nj, t, :].transpose(0, 3, 2, 1)
        out[:, :, j0:j0 + nj, 1::2] = o[:, :, 64 + m0:64 + m0 + nj, t, :].transpose(
            0, 3, 2, 1
        )
    return out


_NC_CACHE = {}


def _get_nc():
    if "nc" not in _NC_CACHE:
        nc = build_nc()
        nc.finalize()
        _NC_CACHE["nc"] = nc
    return _NC_CACHE["nc"]


def run(x, kernel, trace=False, **kw):
    assert x.shape == (B, H, W, C) and kernel.shape == (C, K, K)
    nc = _get_nc()
    xdev = _host_x(np.asarray(x, dtype=np.float32))
    wdev = _host_w(np.asarray(kernel))
    in_maps = [{"x": xdev[b], "w": wdev} for b in range(NCORES)]
    res = run_bass_kernel_spmd(nc, in_maps, list(range(NCORES)), trace=trace, **kw)
    odev = np.stack([np.asarray(res.results[b]["out"]) for b in range(NCORES)])
    return _host_unshuffle(odev), res


def kernel(x, kernel):
    out, _ = run(np.asarray(x), np.asarray(kernel))
    return out
